# revision 2
# baseline (speedup 1.0000x reference)
"""Trainium2 Bass kernel for EuclideanSimilarity (retrieval_knn), v2.

Reference per batch b (B=8, L=4096, D=128, LQ=2048):
    proj = x @ W.T + b                        [L, D]
    q    = avgpool2(x) @ W.T + b              [LQ, D] (== avgpool2(proj))
    power= ||q_i||^2 + ||k_j||^2 - 2 q_i.k_j  [LQ, L]
    sim  = exp(-sqrt(power))
    k    = sim @ proj                         [LQ, D]
    returns (q, k, v=k)

One batch element per core. The kernel is ACT(scalar-engine)-bound: sqrt+exp
over 8.4M elements is 2 passes at 1 elem/lane/cycle @1.2GHz (~110us floor).
Everything else is arranged to stay off ACT and under that floor:

  - GEMM2 (-2qk) accumulates `qsq_i + ksq_j` in PSUM via an appended K=2
    "rank-2" matmul ([ones;ksq_jt]^T @ [qsq;ones]) -- PE cost is per output
    column, so the extra pass costs 512 cycles/tile and the DVE power-build
    pass of the old kernel disappears entirely.
  - ACT sqrt reads the GEMM2 PSUM tiles directly (4-bank [128,2048] ops) and
    writes an fp16 dist matrix [128, 65536] in SBUF.  All 32 sqrt ops run
    back-to-back (one table set), then all 16 exp ops (one table set):
    exactly 2 ACT table loads per kernel after the initial one.
  - exp writes bf16 sim strips INTO the already-consumed region of the dist
    tile (bitcast f16->bf16, one strip behind) so the 128KB dist allocation
    is reused and SBUF stays under budget.
  - ksq/qsq rows come from ones-matmul partition reductions sliced at PSUM
    partition 0.  Engine writes must start at partition 0, so the ones rows
    of the rank-2 operands are DMA-initialized from DRAM and the ksq row is
    placed on partition 1 of its tile by a single SBUF->SBUF DMA.
  - qT is pooled from projTm2 in SBUF (projTm2 = -2(Wx+b), so
    -0.25*(pm2_2i+pm2_2i+1) = avgpool(Wx+b) = q exactly, bias included).
  - all f32 data is declared float32r (same bits) so every matmul streams at
    1 cycle/column; projnat/sim are bf16 (GEMM3 operands).

Host pre/post: x transposed per batch, qT/kT transposed back (free wrt HW).
"""

import os
import sys

for _p in ("/opt/trn_rl_repo", "/root/.axon_site/_ro/trn_rl_repo"):
    if os.path.isdir(_p) and _p not in sys.path:
        sys.path.insert(0, _p)

import numpy as np

import concourse.bass as bass
import concourse.mybir as mybir
from concourse import bacc
from concourse.bass_utils import run_bass_kernel_spmd
from concourse.tile import TileContext
from concourse.tile_rust import add_dep_helper

B, L, D = 8, 4096, 128
LQ = L // 2
P = 128
NJT = L // P          # 32 j-tiles
NCH = LQ // 512       # 4 query chunks of 512
F32 = mybir.dt.float32
F32R = mybir.dt.float32r
F16 = mybir.dt.float16
BF16 = mybir.dt.bfloat16

AF = mybir.ActivationFunctionType
ALU = mybir.AluOpType


def build_nc(repeat=1):
    nc = bacc.Bacc("TRN2", target_bir_lowering=False)

    xT = nc.declare_dram_parameter("xT", [P, L], F32R, isOutput=False)
    WT = nc.declare_dram_parameter("WT", [P, D], F32R, isOutput=False)
    Wm2T = nc.declare_dram_parameter("Wm2T", [P, D], F32R, isOutput=False)
    bm2 = nc.declare_dram_parameter("bm2", [P, 1], F32, isOutput=False)
    bbc4 = nc.declare_dram_parameter("bbc4", [P, 512], F32, isOutput=False)
    ones_in = nc.declare_dram_parameter("ones_mat", [P, P], F32R, isOutput=False)
    # rank-2 operand seeds: row1 = ones (row0 overwritten on-device)
    rks_in = nc.declare_dram_parameter("rks_init", [2, L], F32R, isOutput=False)
    rkm_in = nc.declare_dram_parameter("rkm_init", [2, LQ], F32R, isOutput=False)

    qT_out = nc.declare_dram_parameter("qT", [P, LQ], F32R, isOutput=True)
    kT_out = nc.declare_dram_parameter("kT", [P, LQ], F32, isOutput=True)

    with TileContext(nc) as tc:
      for _rep in range(repeat):
        act_chain = {"prev": None}

        def act(*args, **kwargs):
            s = nc.scalar.activation(*args, **kwargs)
            if act_chain["prev"] is not None:
                add_dep_helper(s.ins, act_chain["prev"].ins, sync=False,
                               reason="act order chain")
            act_chain["prev"] = s
            return s

        with (
            tc.tile_pool(name="big", bufs=1) as big,
            tc.tile_pool(name="work", bufs=2) as work,
        ):
            WT_sb = big.tile([P, D], F32R)
            Wm2T_sb = big.tile([P, D], F32R)
            bm2_sb = big.tile([P, 1], F32)
            bbc4_sb = big.tile([P, 512], F32)
            ones_sb = big.tile([P, P], F32R)
            projnat = big.tile([P, L], BF16)      # [j, e] GEMM3 stationary
            qT_sb = big.tile([P, LQ], F32R)       # [e, i]
            rk_stat = big.tile([2, L], F32R)      # [ones; ksq_j] (stationary)
            rk_mov = big.tile([2, LQ], F32R)      # [qsq_i; ones] (moving)

            # All consts ride the ACT queue (critical ones first) so the SP
            # queue belongs to the xT blocks and Pool stays free
            nc.scalar.dma_start(out=Wm2T_sb[:], in_=Wm2T[:])
            nc.scalar.dma_start(out=bm2_sb[:], in_=bm2[:])
            nc.scalar.dma_start(out=ones_sb[:], in_=ones_in[:])
            nc.gpsimd.dma_start(out=WT_sb[:], in_=WT[:])
            nc.gpsimd.dma_start(out=bbc4_sb[:], in_=bbc4[:])
            nc.gpsimd.dma_start(out=rk_stat[:], in_=rks_in[:])
            nc.gpsimd.dma_start(out=rk_mov[:], in_=rkm_in[:])
            bm2_col = bm2_sb[:, 0:1]

            with tc.tile_pool(name="pjp", bufs=1) as pjp:
                projTm2 = pjp.tile([P, L], F32R)  # [e, l] = -2(Wx+b)^T

                # ---------------- phase 1 ----------------
                with (
                    tc.tile_pool(name="ph1", bufs=1) as ph1,
                    tc.tile_pool(name="psA", bufs=3, space="PSUM") as psA,
                    tc.tile_pool(name="psB", bufs=2, space="PSUM") as psB,
                ):
                    xT_sb = ph1.tile([P, L], F32R)
                    ksq_row = ph1.tile([1, L], F32R)  # staging on partition 0
                    for c in range(8):
                        nc.sync.dma_start(
                            out=xT_sb[:, c * 512:(c + 1) * 512],
                            in_=xT[:, c * 512:(c + 1) * 512])

                    # Per 512-block: GEMM1 psum -> ACT drain (identity+bias)
                    # -> DVE qT pooling (from SBUF) -> Pool square -> ones-mm
                    # -> ACT ksq row (partition-0 staging).  The row ACT ops
                    # are emitted LAGGED (2 blocks behind the drains) so the
                    # ACT in-order queue never stalls on the Pool/PE round
                    # trip of the current block.
                    row_backlog = []      # (kind, idx) pending ACT row ops
                    ones_ps = {}          # idx -> psum tile
                    qsq_ps = {}

                    def emit_row(kind, idx):
                        if kind == "ksq":
                            blk = slice(idx * 512, (idx + 1) * 512)
                            act(ksq_row[0:1, blk], ones_ps.pop(idx)[0:1, :],
                                AF.Identity, scale=0.25)
                            nc.sync.dma_start(out=rk_stat[1:2, blk],
                                              in_=ksq_row[0:1, blk])
                        else:
                            qblk = slice(idx * 512, (idx + 1) * 512)
                            act(rk_mov[0:1, qblk], qsq_ps.pop(idx)[0:1, :],
                                AF.Identity)

                    def pop_row():
                        return row_backlog.pop(0)

                    for c in range(8):
                        blk = slice(c * 512, (c + 1) * 512)
                        ps = psA.tile([P, 512], F32, tag="ps1")
                        nc.tensor.matmul(ps, Wm2T_sb[:], xT_sb[:, blk],
                                         start=True, stop=True)
                        act(projTm2[:, blk], ps, AF.Identity, bias=bm2_col)
                        # qT = -0.25*(pm2_{2i} + pm2_{2i+1})  (bias included)
                        pm = projTm2[:, blk].rearrange(
                            "p (i two) -> p i two", two=2)
                        qtmp = work.tile([P, 256], F32, tag="qtmp")
                        nc.vector.tensor_add(qtmp[:], pm[:, :, 0], pm[:, :, 1])
                        nc.vector.tensor_scalar_mul(
                            qT_sb[:, c * 256:(c + 1) * 256], qtmp[:], -0.25)
                        # ksq row: 0.25 * colsum(projTm2_blk^2)
                        sq = work.tile([P, 512], F32R, tag="sq")
                        nc.gpsimd.tensor_mul(sq[:], projTm2[:, blk],
                                             projTm2[:, blk])
                        ps2 = psB.tile([P, 512], F32, tag="ps2")
                        nc.tensor.matmul(ps2, ones_sb[:], sq[:],
                                         start=True, stop=True)
                        ones_ps[c] = ps2
                        row_backlog.append(("ksq", c))
                        # qsq row (direct partition-0 write into rk_mov)
                        if c % 2 == 1:
                            qc = (c - 1) // 2
                            qblk = slice(qc * 512, (qc + 1) * 512)
                            sqq = work.tile([P, 512], F32R, tag="sqq")
                            nc.vector.tensor_mul(sqq[:], qT_sb[:, qblk],
                                                 qT_sb[:, qblk])
                            ps3 = psB.tile([P, 512], F32, tag="ps3")
                            nc.tensor.matmul(ps3, ones_sb[:], sqq[:],
                                             start=True, stop=True)
                            qsq_ps[qc] = ps3
                            row_backlog.append(("qsq", qc))
                        while len(row_backlog) > 2:
                            emit_row(*pop_row())
                    while row_backlog:
                        emit_row(*pop_row())
                    nc.sync.dma_start(out=qT_out[:], in_=qT_sb[:])

                    # projnat[l, e] = x_tile^T W^T + b, bf16
                    for g in range(8):
                        psn = psA.tile([P, 512], F32, tag="ps1")
                        for k in range(4):
                            tt = g * 4 + k
                            nc.tensor.matmul(
                                psn[:, k * 128:(k + 1) * 128],
                                xT_sb[:, tt * 128:(tt + 1) * 128], WT_sb[:],
                                start=True, stop=True)
                        nc.vector.tensor_add(
                            projnat[:, g * 512:(g + 1) * 512], psn,
                            bbc4_sb[:])

                # ---------------- phase A: GEMM2 + sqrt ----------------
                qTr = qT_sb[:]
                with tc.tile_pool(name="distp", bufs=1) as distp:
                    dist = distp.tile([P, L * LQ // P], F16)   # [128, 65536]
                    sim0 = distp.tile([P, 4096], BF16)
                    with tc.tile_pool(name="g2", bufs=2, space="PSUM") as g2:
                        for jt in range(NJT):
                            pst = g2.tile([P, 2048], F32, tag="g2")
                            jb = slice(jt * 128, (jt + 1) * 128)
                            for c in range(NCH):
                                ib = slice(c * 512, (c + 1) * 512)
                                nc.tensor.matmul(
                                    pst[:, ib], projTm2[:, jb], qTr[:, ib],
                                    start=True, stop=False)
                                nc.tensor.matmul(
                                    pst[:, ib], rk_stat[0:2, jb],
                                    rk_mov[0:2, ib],
                                    start=False, stop=True)
                            act(dist[:, jt * 2048:(jt + 1) * 2048], pst[:],
                                AF.Sqrt)

                    # ---------------- phase B: exp + GEMM3 ----------------
                    # Strip order [15, 0, 1, .., 14]: u15 lands in sim0 and
                    # frees its dist region for u0, so every exp writes into
                    # the region consumed by the PREVIOUS strip (bitcast
                    # f16->bf16).  GEMM3 start/stop flags follow per-slice
                    # emission counters; kT drains per chunk.
                    with tc.tile_pool(name="kps", bufs=1, space="PSUM") as kps:
                        kpst = kps.tile([P, 2048], F32)
                        kT_sb = sim0[:].bitcast(F32)
                        ndone = [0] * NCH
                        order = [15] + list(range(15))
                        for pos, u in enumerate(order):
                            ub = slice(u * 4096, (u + 1) * 4096)
                            if pos == 0:
                                sim = sim0[:]
                            else:
                                prev = order[pos - 1]
                                sim = dist[:, prev * 4096:(prev + 1) * 4096] \
                                    .bitcast(BF16)
                            act(sim, dist[:, ub], AF.Exp, scale=-1.0)
                            for jj in range(2):
                                jt = 2 * u + jj
                                for c in range(NCH):
                                    nc.tensor.matmul(
                                        kpst[:, c * 512:(c + 1) * 512],
                                        projnat[:, jt * 128:(jt + 1) * 128],
                                        sim[:, jj * 2048 + c * 512:
                                            jj * 2048 + (c + 1) * 512],
                                        start=(ndone[c] == 0),
                                        stop=(ndone[c] == NJT - 1))
                                    ndone[c] += 1
                                    if ndone[c] == NJT:
                                        cb = slice(c * 512, (c + 1) * 512)
                                        nc.vector.tensor_copy(
                                            kT_sb[:, cb], kpst[:, cb])
                                        nc.sync.dma_start(
                                            out=kT_out[:, cb],
                                            in_=kT_sb[:, cb])

    nc.compile()
    return nc


_NC_CACHE = {}


def _get_nc():
    if "nc" not in _NC_CACHE:
        _NC_CACHE["nc"] = build_nc()
    return _NC_CACHE["nc"]


def make_in_maps(x, W, b):
    x = np.asarray(x, np.float32)
    W = np.asarray(W, np.float32)
    b = np.asarray(b, np.float32)
    WT = np.ascontiguousarray(W.T)
    Wm2T = np.ascontiguousarray((-2.0 * W).T)
    bm2 = np.ascontiguousarray((-2.0 * b).reshape(P, 1))
    bbc4 = np.ascontiguousarray(np.tile(b.reshape(1, D), (P, 4))
                                ).astype(np.float32)
    ones_mat = np.ones((P, P), np.float32)
    rks_init = np.zeros((2, L), np.float32)
    rks_init[0, :] = 1.0          # row0 = ones (row1 = ksq, written on-device)
    rkm_init = np.zeros((2, LQ), np.float32)
    rkm_init[1, :] = 1.0          # row1 = ones (row0 = qsq, written on-device)
    return [{
        "xT": np.ascontiguousarray(x[i].T),
        "WT": WT, "Wm2T": Wm2T, "bm2": bm2, "bbc4": bbc4,
        "ones_mat": ones_mat, "rks_init": rks_init, "rkm_init": rkm_init,
    } for i in range(B)]


def kernel(x, W, b):
    x = np.asarray(x, dtype=np.float32)
    W = np.asarray(W, dtype=np.float32)
    b = np.asarray(b, dtype=np.float32)

    nc = _get_nc()
    in_maps = make_in_maps(x, W, b)

    trace = bool(int(os.environ.get("KBENCH_TRACE", "0")))
    kres = None
    last_exc = None
    for attempt in range(5):
        try:
            kres = run_bass_kernel_spmd(nc, in_maps, list(range(B)), trace=trace)
            break
        except Exception as exc:
            last_exc = exc
            import time as _time
            _time.sleep(3.0 * (attempt + 1))
    if kres is None:
        raise last_exc
    _NC_CACHE["last_result"] = kres
    res = kres.results

    q = np.stack([np.ascontiguousarray(r["qT"].T) for r in res])
    k = np.stack([np.ascontiguousarray(r["kT"].T) for r in res])
    return q, k, k


# revision 3
# speedup vs baseline: 1.8732x; 1.8732x over previous
"""Trainium2 Bass kernel for EuclideanSimilarity (retrieval_knn).

Per-core pipeline (one batch element per NeuronCore, 8 cores):
  projT_m2[e,l] = (-2W)^T x + (-2b)      8 matmuls, stationary -2W^T
  qT[e,i]       = -0.25*(psum pooled)    pooled off the fp32 GEMM1 PSUM
  projnat[l,e]  = x_tile^T W^T + b       32 matmuls, xT tiles stationary
  ksq[j]        = sum_e projnat^2        one ACT Square+accum per tile
  qsq_bcast     = ones^T @ qT^2          reduce+partition-broadcast matmul
  per 512-query chunk (software-pipelined; GEMM2/GEMM3 operands f32r):
    psum  = -2 q.k ; power = psum + ksq[j] + qsq[i] (fused DVE)
    sim   = Exp(-Sqrt(power)) on ACT; kT += projnat_jt @ sim (GEMM3)
"""

import os
import sys

for _p in ("/opt/trn_rl_repo", "/root/.axon_site/_ro/trn_rl_repo"):
    if os.path.isdir(_p) and _p not in sys.path:
        sys.path.insert(0, _p)

import numpy as np

import concourse.bass as bass
import concourse.mybir as mybir
from concourse import bacc
from concourse.bass_utils import run_bass_kernel_spmd
from concourse.tile import TileContext
from concourse.tile_rust import add_dep_helper

B, L, D = 8, 4096, 128
LQ = L // 2
P = 128
NI = 512
NCHUNK = LQ // NI
NJT = L // P
F32 = mybir.dt.float32
F32R = mybir.dt.float32r

KMODE = os.environ.get("KMODE", "f32r")

AF = mybir.ActivationFunctionType
ALU = mybir.AluOpType


def build_nc(repeat=1, mode=None):
    mode = KMODE if mode is None else mode
    g2r = mode in ("f32r", "f32r2")
    g3r = mode == "f32r"
    G2DT = F32R if g2r else F32
    G3DT = F32R if g3r else F32
    nc = bacc.Bacc("TRN2", target_bir_lowering=False)

    xT = nc.declare_dram_parameter("xT", [P, L], F32, isOutput=False)
    WT = nc.declare_dram_parameter("WT", [P, D], F32, isOutput=False)
    Wm2T = nc.declare_dram_parameter("Wm2T", [P, D], F32, isOutput=False)
    bcols = nc.declare_dram_parameter("bcols", [P, 2], F32, isOutput=False)
    b_bcast_in = nc.declare_dram_parameter("b_bcast", [P, D], F32, isOutput=False)
    ones_in = nc.declare_dram_parameter("ones_mat", [P, P], F32, isOutput=False)

    qT_out = nc.declare_dram_parameter("qT", [P, LQ], F32, isOutput=True)
    kT_out = nc.declare_dram_parameter("kT", [P, LQ], F32, isOutput=True)

    with TileContext(nc) as tc:
      for _rep in range(repeat):
        with (
            tc.tile_pool(name="consts", bufs=1) as consts,
            tc.tile_pool(name="big", bufs=1) as big,
            tc.tile_pool(name="work", bufs=4) as work,
            tc.tile_pool(name="ps1", bufs=4, space="PSUM") as ps1,
        ):
            WT_sb = consts.tile([P, D], F32)
            Wm2T_sb = consts.tile([P, D], F32)
            bcols_sb = consts.tile([P, 2], F32)
            b_bcast = consts.tile([P, D], F32)
            ones_sb = consts.tile([P, P], F32)
            nc.sync.dma_start(out=WT_sb[:], in_=WT[:])
            nc.sync.dma_start(out=Wm2T_sb[:], in_=Wm2T[:])
            nc.sync.dma_start(out=bcols_sb[:], in_=bcols[:])
            nc.sync.dma_start(out=b_bcast[:], in_=b_bcast_in[:])
            nc.sync.dma_start(out=ones_sb[:], in_=ones_in[:])
            b_col = bcols_sb[:, 0:1]
            bm2_col = bcols_sb[:, 1:2]

            projTm2 = big.tile([P, L], G2DT)
            projnat = big.tile([P, L], G3DT)
            if g2r:
                qT_mm = big.tile([P, LQ], G2DT, tag="qT_mm", name="qT_mm")
            qsq_bcast = big.tile([P, LQ], F32)
            ksq = consts.tile([P, NJT], F32)

            with tc.tile_pool(name="phase1", bufs=1) as ph1:
                xT_sb = ph1.tile([P, L], F32)
                if g2r:
                    qT_sb = ph1.tile([P, LQ], F32, tag="qT_sb", name="qT_sb")
                else:
                    qT_sb = big.tile([P, LQ], F32, tag="qT_sb", name="qT_sb")
                    qT_mm = qT_sb
                for c in range(L // 512):
                    nc.sync.dma_start(
                        out=xT_sb[:, c * 512:(c + 1) * 512],
                        in_=xT[:, c * 512:(c + 1) * 512])

                for c in range(L // 512):
                    ps = ps1.tile([P, 512], F32, tag="ps1")
                    nc.tensor.matmul(
                        ps, Wm2T_sb[:], xT_sb[:, c * 512:(c + 1) * 512],
                        start=True, stop=True,
                    )
                    if g2r:
                        src32 = work.tile([P, 512], F32, tag="pm2f32")
                        nc.vector.tensor_scalar_add(src32[:], ps, bm2_col)
                        nc.vector.tensor_copy(
                            projTm2[:, c * 512:(c + 1) * 512], src32[:])
                        src32 = src32[:]
                    else:
                        src32 = projTm2[:, c * 512:(c + 1) * 512]
                        nc.vector.tensor_scalar_add(src32, ps, bm2_col)
                    sp = src32.rearrange("p (i two) -> p i two", two=2)
                    qtmp = work.tile([P, 256], F32, tag="qtmp")
                    nc.vector.tensor_add(qtmp[:], sp[:, :, 0], sp[:, :, 1])
                    nc.vector.tensor_scalar_mul(
                        qT_sb[:, c * 256:(c + 1) * 256], qtmp[:], -0.25)
                nc.sync.dma_start(out=qT_out[:], in_=qT_sb[:])
                if g2r:
                    nc.gpsimd.tensor_copy(qT_mm[:], qT_sb[:])

                for t in range(NJT):
                    ps = ps1.tile([P, D], F32, tag="ps1")
                    nc.tensor.matmul(
                        ps, xT_sb[:, t * P:(t + 1) * P], WT_sb[:],
                        start=True, stop=True,
                    )
                    if g3r:
                        seg32 = work.tile([P, D], F32, tag="sqs")
                        nc.vector.tensor_add(seg32[:], ps, b_bcast[:])
                        nc.vector.tensor_copy(
                            projnat[:, t * P:(t + 1) * P], seg32[:])
                    else:
                        seg32 = projnat[:, t * P:(t + 1) * P]
                        nc.vector.tensor_add(seg32, ps, b_bcast[:])
                    sq = work.tile([P, D], F32, tag="sqs")
                    nc.scalar.activation(
                        sq[:], seg32[:], AF.Square,
                        accum_out=ksq[:, t:t + 1])

                sq_qT = ph1.tile([P, LQ], F32)
                nc.gpsimd.tensor_mul(sq_qT[:], qT_sb[:], qT_sb[:])
                for c in range(LQ // 512):
                    ps = ps1.tile([P, 512], F32, tag="ps1")
                    nc.tensor.matmul(
                        ps, ones_sb[:], sq_qT[:, c * 512:(c + 1) * 512],
                        start=True, stop=True,
                    )
                    nc.scalar.copy(qsq_bcast[:, c * 512:(c + 1) * 512], ps)

            NQ = 8
            QJT = NJT // NQ
            with (
                tc.tile_pool(name="stripp", bufs=NQ) as stripp,
                tc.tile_pool(name="simp", bufs=1) as simp,
                tc.tile_pool(name="psqk", bufs=3, space="PSUM") as psqk,
                tc.tile_pool(name="psk", bufs=1, space="PSUM") as psk,
            ):
                state = {}
                last_exp = {"i": None}

                def emit_power_sqrt(c):
                    qs = qsq_bcast[:, c * NI:(c + 1) * NI]
                    qchunk = qT_mm[:, c * NI:(c + 1) * NI]
                    sim = simp.tile([P, NJT * NI], G3DT, tag="sim", name="sim")
                    quarters = []
                    for h in range(NQ):
                        power = stripp.tile(
                            [P, QJT * NI], F32, tag="power", name="power")
                        quarters.append(power)
                        for j in range(QJT):
                            jt = h * QJT + j
                            ps2 = psqk.tile([P, NI], F32, tag="qk")
                            nc.tensor.matmul(
                                ps2, projTm2[:, jt * P:(jt + 1) * P], qchunk,
                                start=True, stop=True,
                            )
                            nc.vector.affine_then_add(
                                power[:, j * NI:(j + 1) * NI], ps2, qs,
                                scale=1.0, bias=ksq[:, jt:jt + 1],
                            )
                    sqrt_last = None
                    for h in range(NQ):
                        s = nc.scalar.activation(
                            quarters[h][:], quarters[h][:], AF.Sqrt)
                        prev = sqrt_last if h else last_exp["i"]
                        if prev is not None:
                            add_dep_helper(
                                s.ins, prev.ins, sync=False,
                                reason="act set batch: sqrt chain")
                        sqrt_last = s
                    state[c] = (quarters, sim, sqrt_last)

                def emit_exp_gemm3(c):
                    quarters, sim, sqrt_last = state.pop(c)
                    ps3 = psk.tile([P, NI], F32, tag="kacc")
                    for h in range(NQ):
                        e = nc.scalar.activation(
                            sim[:, h * QJT * NI:(h + 1) * QJT * NI],
                            quarters[h][:], AF.Exp, scale=-1.0)
                        prev = last_exp["i"] if h else sqrt_last
                        add_dep_helper(
                            e.ins, prev.ins, sync=False,
                            reason="act set batch: exp chain")
                        last_exp["i"] = e
                        for j in range(QJT):
                            jt = h * QJT + j
                            nc.tensor.matmul(
                                ps3, projnat[:, jt * P:(jt + 1) * P],
                                sim[:, jt * NI:(jt + 1) * NI],
                                start=(jt == 0), stop=(jt == NJT - 1),
                            )
                    kT_tile = work.tile([P, NI], F32, tag="kout")
                    nc.vector.tensor_copy(kT_tile[:], ps3)
                    nc.sync.dma_start(
                        out=kT_out[:, c * NI:(c + 1) * NI], in_=kT_tile[:])

                for c in range(NCHUNK):
                    if c >= 1:
                        emit_exp_gemm3(c - 1)
                    emit_power_sqrt(c)
                emit_exp_gemm3(NCHUNK - 1)

    nc.compile()
    return nc


def make_in_maps(x, W, b):
    x = np.asarray(x, np.float32)
    W = np.asarray(W, np.float32)
    b = np.asarray(b, np.float32)
    WT = np.ascontiguousarray(W.T)
    Wm2T = np.ascontiguousarray((-2.0 * W).T)
    bcols = np.stack([b, -2.0 * b], axis=1).astype(np.float32)
    b_bcast = np.ascontiguousarray(
        np.broadcast_to(b.reshape(1, D), (P, D)).astype(np.float32))
    ones_mat = np.ones((P, P), np.float32)
    return [{
        "xT": np.ascontiguousarray(x[i].T),
        "WT": WT, "Wm2T": Wm2T, "bcols": bcols, "b_bcast": b_bcast,
        "ones_mat": ones_mat,
    } for i in range(B)]


_NC_CACHE = {}


def _get_nc():
    key = ("nc", KMODE)
    if key not in _NC_CACHE:
        _NC_CACHE[key] = build_nc()
    return _NC_CACHE[key]


def kernel(x, W, b):
    nc = _get_nc()
    in_maps = make_in_maps(x, W, b)

    trace = bool(int(os.environ.get("KBENCH_TRACE", "0")))
    kres = None
    last_exc = None
    for attempt in range(5):
        try:
            kres = run_bass_kernel_spmd(nc, in_maps, list(range(B)), trace=trace)
            break
        except Exception as exc:
            last_exc = exc
            import time as _time
            _time.sleep(3.0 * (attempt + 1))
    if kres is None:
        raise last_exc
    _NC_CACHE["last_result"] = kres
    res = kres.results

    q = np.stack([np.ascontiguousarray(r["qT"].T) for r in res])
    k = np.stack([np.ascontiguousarray(r["kT"].T) for r in res])
    return q, k, k
